# revision 1
# baseline (speedup 1.0000x reference)
"""Trainium2 Bass kernel for nn_DGLFeatureGAT (dense per-batch GAT over F=256
feature-nodes, window length W=256, H=4 heads, D=8).

Strategy (data-parallel over batch, 32 batches/core on 8 cores):

The attention weights are alpha = softmax_s(leaky_relu(el_s + er_i)). Softmax
over s cancels any dst(i)-only factor, so with
    a_s = exp(el_s),  c_s = exp(0.2*el_s),  h_i = exp(0.8*er_i)
the unnormalized weights can be taken as
    Etil[s,i] = max(a_s * h_i, c_s)          (== exp(lrelu(el_s+er_i))/exp(0.2*er_i))
which is ONE fused DVE tensor_scalar op (mult+max with two per-partition
scalars) per [128,256] tile over a partition-broadcast h row — no dense
transcendentals, no dense tensor_tensor.

Per (batch, head) the aggregation sum_s Etil[s,i]*[feat_h(s,:)|1] runs on
TensorE with a zero-padded M=32 lhsT (so all PSUM rows are written), giving
numerator rows and the denominator row; normalization uses a selection-matmul
denominator broadcast + fast reciprocal + one tensor_tensor multiply; the
final projection is a single K=128 matmul against a zero-padded proj_w whose
"ones row" carries the fused bias term pb2 = gat_bias @ proj_w.T + proj_b.
"""
import sys
import numpy as np

sys.path.insert(0, "/opt/trn_rl_repo")

import concourse.bass as bass  # noqa: E402
import concourse.bacc as bacc  # noqa: E402
import concourse.tile as tile  # noqa: E402
from concourse import mybir  # noqa: E402
from concourse.bass_utils import run_bass_kernel_spmd  # noqa: E402

F32 = mybir.dt.float32
BF16 = mybir.dt.bfloat16

B, W, F, H, D = 256, 256, 256, 4, 8
O = H * D  # 32
NCORES = 8
NB = B // NCORES  # 32 batches per core
NEG_SLOPE = 0.2


def build_nc(nb: int = NB, trace_scopes: bool = False):
    """Build the single-core Bass program processing `nb` batches."""
    assert nb % 4 == 0
    nc = bacc.Bacc("TRN2", target_bir_lowering=False, debug=False)

    x = nc.dram_tensor("x", [nb, W, F], F32, kind="ExternalInput")
    c36 = nc.dram_tensor("c36", [W, 36], F32, kind="ExternalInput")
    fcr4 = nc.dram_tensor("fcr4", [W, 4], F32, kind="ExternalInput")
    sel2r = nc.dram_tensor("sel2r", [128, 128], F32, kind="ExternalInput")
    pwt = nc.dram_tensor("pwt", [128, W], F32, kind="ExternalInput")
    y = nc.dram_tensor("y", [nb, W, F], F32, kind="ExternalOutput")

    # fixed ping-pong featpad buffers, ones/zero cols initialized once
    featpads = [
        nc.alloc_sbuf_tensor(f"featpad{i}", [128, 256], BF16) for i in range(2)
    ]

    with tile.TileContext(nc) as tc:
        with (
            tc.tile_pool(name="consts", bufs=1) as cpool,
            tc.tile_pool(name="xp", bufs=2) as xpool,
            tc.tile_pool(name="yp", bufs=2) as ypool,
            tc.tile_pool(name="hb", bufs=10) as hbpool,
            tc.tile_pool(name="et", bufs=10) as etpool,
            tc.tile_pool(name="h4", bufs=2) as h4pool,
            tc.tile_pool(name="hr", bufs=8) as hrpool,
            tc.tile_pool(name="ac", bufs=2) as acpool,
            tc.tile_pool(name="nm", bufs=2) as nmpool,
            tc.tile_pool(name="psA", bufs=2, space="PSUM") as psA,
            tc.tile_pool(name="psE", bufs=1, space="PSUM") as psE,
            tc.tile_pool(name="psG", bufs=2, space="PSUM") as psG,
            tc.tile_pool(name="psD", bufs=1, space="PSUM") as psD,
            tc.tile_pool(name="psO", bufs=2, space="PSUM") as psO,
        ):
            # ---- constants into SBUF
            c36_sb = cpool.tile([128, 72], F32)
            nc.sync.dma_start(
                c36_sb[:].rearrange("p (c n) -> p c n", c=2),
                c36.ap().rearrange("(c p) n -> p c n", c=2),
            )
            fcr_sb = cpool.tile([128, 8], F32)
            nc.sync.dma_start(
                fcr_sb[:].rearrange("p (c n) -> p c n", c=2),
                fcr4.ap().rearrange("(c p) n -> p c n", c=2),
            )
            sel2r_sb = cpool.tile([128, 128], F32)
            nc.sync.dma_start(sel2r_sb[:], sel2r.ap())
            pwt_sb = cpool.tile([128, 256], F32)
            nc.sync.dma_start(pwt_sb[:], pwt.ap())

            # featpad init: zeros everywhere, ones at col {128*sc + 32*h + 8}
            for fp in featpads:
                nc.vector.memset(fp.ap(), 0.0)
                ones_view = fp.ap().rearrange(
                    "q (sc h j) -> q sc h j", sc=2, h=4
                )[:, :, :, 8:9]
                nc.vector.memset(ones_view, 1.0)

            x_t = None
            y_t = None
            for g in range(nb // 2):
                if g % 2 == 0:
                    # x staging layout: col = 1024*kc + 256*(b%4) + f
                    b0 = 2 * g
                    x_t = xpool.tile([128, 2048], F32, tag="xt")
                    for kc in range(2):
                        nc.sync.dma_start(
                            x_t[:, 1024 * kc : 1024 * kc + 1024].rearrange(
                                "q (b f) -> q b f", b=4
                            ),
                            x.ap()[b0 : b0 + 4].rearrange(
                                "b (kc q) f -> kc q b f", kc=2
                            )[kc],
                        )
                    y_t = ypool.tile([128, 2048], F32, tag="yt")

                # ---------------- per-group tiles
                formA = psA.tile([128, 144], F32, tag="formA")
                er_ps = psE.tile([128, 512], F32, tag="er")
                agg = psG.tile([128, 512], F32, tag="agg")
                den = psD.tile([128, 512], F32, tag="den")
                ac_sb = acpool.tile([128, 32], F32, tag="ac")
                num_sb = nmpool.tile([128, 512], F32, tag="num")
                rinv = nmpool.tile([128, 512], F32, tag="rinv")
                num_sc = nmpool.tile([128, 512], F32, tag="nsc")

                bm = [(2 * g + p) % 4 for p in range(2)]
                yoff = [512 * bm[p] for p in range(2)]

                # ---------------- stage 1: feat/el (formA) + er (formB) matmuls
                for p in range(2):
                    for fh in range(2):
                        for kc in range(2):
                            xo = 1024 * kc + 256 * bm[p] + 128 * fh
                            nc.tensor.matmul(
                                formA[:, 72 * p + 36 * fh : 72 * p + 36 * fh + 36],
                                x_t[:, xo : xo + 128],
                                c36_sb[:, 36 * kc : 36 * kc + 36],
                                start=(kc == 0),
                                stop=(kc == 1),
                            )
                for p in range(2):
                    for kc in range(2):
                        xo = 1024 * kc + 256 * bm[p]
                        nc.tensor.matmul(
                            er_ps[0:4, 256 * p : 256 * p + 256],
                            fcr_sb[:, 4 * kc : 4 * kc + 4],
                            x_t[:, xo : xo + 256],
                            start=(kc == 0),
                            stop=(kc == 1),
                        )

                # ---------------- stage 2: ACT exps + feat scatter
                el_view = formA[:].rearrange("q (p fh n) -> q p fh n", p=2, fh=2)[
                    :, :, :, 32:36
                ]
                a_view = ac_sb[:, 0:16].rearrange("q (p fh n) -> q p fh n", p=2, fh=2)
                c_view = ac_sb[:, 16:32].rearrange("q (p fh n) -> q p fh n", p=2, fh=2)
                nc.scalar.activation(
                    a_view, el_view, mybir.ActivationFunctionType.Exp, scale=1.0
                )
                nc.scalar.activation(
                    c_view, el_view, mybir.ActivationFunctionType.Exp, scale=NEG_SLOPE
                )
                # h rows: exp(0.8 * er) on rows 0..3; HW partition_broadcast
                # only reads partition 0 at offset 0, so DMA each head row
                # into its own [1, 512] tile (both batches of the group).
                h4d = h4pool.tile([4, 512], BF16, tag="h4d")
                nc.scalar.activation(
                    h4d[:],
                    er_ps[0:4, :],
                    mybir.ActivationFunctionType.Exp,
                    scale=1.0 - NEG_SLOPE,
                )
                h_rows = []
                for h in range(4):
                    hr = hrpool.tile([1, 512], BF16, tag="hr")
                    nc.sync.dma_start(hr[:], h4d[h : h + 1, :])
                    h_rows.append(hr)
                featpad_g = [featpads[(2 * g + p) % 2] for p in range(2)]
                for p in range(2):
                    fa = formA[:, 72 * p : 72 * p + 72].rearrange(
                        "q (fh n) -> q fh n", fh=2
                    )[:, :, 0:32].rearrange("q fh (h d) -> q fh h d", h=4)
                    fp_view = featpad_g[p].ap().rearrange(
                        "q (sc h j) -> q sc h j", sc=2, h=4
                    )[:, :, :, 0:8]
                    nc.scalar.copy(fp_view, fa)

                # ---------------- stage 3: h broadcast + dense scores + agg
                hb2 = {}
                for h in range(4):
                    hb = hbpool.tile([128, 512], BF16, tag="hb")
                    nc.gpsimd.partition_broadcast(hb[:], h_rows[h][:])
                    hb2[h] = hb
                for p in range(2):
                    for h in range(4):
                        for sc in range(2):
                            et = etpool.tile([128, 256], BF16, tag="et")
                            nc.vector.tensor_scalar(
                                et[:],
                                hb2[h][:, 256 * p : 256 * p + 256],
                                ac_sb[:, 8 * p + 4 * sc + h : 8 * p + 4 * sc + h + 1],
                                ac_sb[:, 16 + 8 * p + 4 * sc + h :][:, :1],
                                mybir.AluOpType.mult,
                                mybir.AluOpType.max,
                            )
                            nc.tensor.matmul(
                                agg[32 * h : 32 * h + 32, 256 * p : 256 * p + 256],
                                featpad_g[p].ap()[
                                    :, 128 * sc + 32 * h : 128 * sc + 32 * h + 32
                                ],
                                et[:],
                                start=(sc == 0),
                                stop=(sc == 1),
                                tile_position=(0, 32 * h),
                            )

                # ---------------- stage 4: normalize + project + store
                nc.scalar.copy(num_sb[:], agg[:])
                nc.tensor.matmul(den[:], sel2r_sb[:], num_sb[:], start=True, stop=True)
                nc.vector.reciprocal_approx_fast(rinv[:], den[:])
                nc.vector.tensor_tensor(
                    num_sc[:], num_sb[:], rinv[:], mybir.AluOpType.mult
                )
                for p in range(2):
                    out2 = psO.tile([128, 512], F32, tag="out2")
                    for wc in range(2):
                        nc.tensor.matmul(
                            out2[:, 256 * wc : 256 * wc + 256],
                            pwt_sb[:, 128 * wc : 128 * wc + 128],
                            num_sc[:, 256 * p : 256 * p + 256],
                            start=True,
                            stop=True,
                        )
                    nc.scalar.copy(
                        y_t[:, yoff[p] : yoff[p] + 512], out2[:]
                    )

                if g % 2 == 1:
                    b0 = 2 * g - 2
                    for wc in range(2):
                        nc.sync.dma_start(
                            y.ap()[b0 : b0 + 4].rearrange(
                                "b (wc q) f -> wc q b f", wc=2
                            )[wc],
                            y_t[:].rearrange("q (b wc f) -> wc q b f", b=4, wc=2)[
                                wc
                            ],
                        )

    nc.compile()
    return nc


def host_prep(fc_w, attn_l, attn_r, gat_bias, proj_w, proj_b):
    """Precompute the tiny constant matrices fed to the kernel."""
    fc_w = np.asarray(fc_w, np.float32)
    attn_l = np.asarray(attn_l, np.float32)
    attn_r = np.asarray(attn_r, np.float32)
    gat_bias = np.asarray(gat_bias, np.float32)
    proj_w = np.asarray(proj_w, np.float32)
    proj_b = np.asarray(proj_b, np.float32)

    # c36[w, 0:32] = fc_w.T ; c36[w, 32+h] = sum_d fc_w[h*8+d, w]*attn_l[h, d]
    fcl = np.einsum("hdw,hd->wh", fc_w.reshape(H, D, W), attn_l)
    c36 = np.concatenate([fc_w.T, fcl], axis=1).astype(np.float32)
    fcr4 = np.einsum("hdw,hd->wh", fc_w.reshape(H, D, W), attn_r).astype(np.float32)

    sel2r = np.zeros((128, 128), np.float32)
    for h in range(H):
        sel2r[32 * h + 8, 32 * h : 32 * h + 32] = 1.0

    pb2 = gat_bias @ proj_w.T + proj_b  # [W]
    pwt = np.zeros((128, W), np.float32)
    for h in range(H):
        for j in range(D):
            pwt[32 * h + j, :] = proj_w[:, 8 * h + j]
    pwt[8, :] = pb2
    return c36, fcr4, sel2r, pwt


_CACHE = {}


def run(inputs, trace=False, trace_kwargs=None):
    """Run on 8 NeuronCores; returns (y, BassKernelResults)."""
    x = np.asarray(inputs["x"], np.float32)
    c36, fcr4, sel2r, pwt = host_prep(
        inputs["fc_w"], inputs["attn_l"], inputs["attn_r"],
        inputs["gat_bias"], inputs["proj_w"], inputs["proj_b"],
    )
    if "nc" not in _CACHE:
        _CACHE["nc"] = build_nc(NB)
    nc = _CACHE["nc"]

    in_maps = []
    for c in range(NCORES):
        shard = np.ascontiguousarray(x[c * NB : (c + 1) * NB])
        in_maps.append(
            {"x": shard, "c36": c36, "fcr4": fcr4, "sel2r": sel2r, "pwt": pwt}
        )
    res = run_bass_kernel_spmd(
        nc, in_maps, core_ids=list(range(NCORES)), trace=trace,
        trace_kwargs=trace_kwargs or {},
    )
    # y[b, w, i] already matches the reference's [B, W, F] output layout.
    y = np.concatenate([r["y"] for r in res.results], axis=0)
    return np.ascontiguousarray(y), res


def kernel(**inputs) -> np.ndarray:
    y, _ = run(inputs, trace=False)
    return y

